# revision 1
# baseline (speedup 1.0000x reference)
"""Trainium2 Bass kernel for nn_Actor2ActorAttention (2-layer edge-attention GNN).

Strategy (single SPMD launch on 8 NeuronCores):
  - Host: sort edges by dst, partition dst range across 8 cores (125 dst per
    "window", 50 windows per core), pad each window to a fixed tile count,
    build int16 gather-index streams + per-edge scalar streams.
  - Device, per layer:
      dense phase: feat @ [W_src^T | w_att | W_dst^T] -> per-actor table
        rows [U | a_src | a_dst] (gatherable by edge src) + per-core-shard
        tables of a_dst and V (window-addressable).
      aggregation phase, per window: dma_gather the table rows of each
        edge's src (+ tiny a_dst gather by local dst), compute
        exp(leaky_relu(score)), build per-tile one-hot-times-ex selector
        matrices, and use the tensor engine to segment-reduce payload,
        denominator and rel-term moments in one PSUM accumulation.
        Finalize: agg = (P + qx*Wr0 + qy*Wr1)/denom + [denom>0]*V, relu.
  - AllGather (collective) exchanges layer-1 features between layers.
"""

import os
import sys

sys.path.insert(0, "/opt/trn_rl_repo")

import numpy as np

# ---------------- problem constants (hardcoded per spec) ----------------
N_ACTORS = 50000
D = 128
L = 2
E = 800000
NCORE = 8
DSTW = 125                      # dst actors per window
NWIN = 50                       # windows per core (8*50*125 == 50000)
SHARD = DSTW * NWIN             # 6250 dst actors per core
SHARD_PAD = 6656                # 13*512, keeps dense 4-tile chunks shard-aligned
NPAD = NCORE * SHARD_PAD        # 53248 padded table rows
NDTILE = NPAD // 128            # 416 dense tiles
DCH = 4                         # dense tiles per transpose-load chunk
HALF = 32768                    # int16 gather split point
ROWE = 256                      # fp16 elements per tabU row (512B)

_PROGRAM_CACHE = {}


def _rho(a):
    """Actor id -> padded table row (shards padded to SHARD_PAD rows)."""
    return (a // SHARD) * SHARD_PAD + (a % SHARD)


def _wrap_idx(vals, n):
    """Lay out gather indices: element i at [i%16, i//16], replicated to all
    8 groups of 16 partitions -> [128, n//16] int16."""
    a = np.zeros(n, np.int16)
    a[: len(vals)] = vals
    return np.tile(a.reshape(-1, 16).T, (8, 1))


def _build_plan(src, dst, rel):
    src = np.asarray(src).astype(np.int64)
    dst = np.asarray(dst).astype(np.int64)
    rel = np.asarray(rel).astype(np.float32)

    order = np.argsort(dst, kind="stable")
    ss = _rho(src[order])
    sd = dst[order]
    sr = rel[order]

    g_of = sd // DSTW
    bounds = np.searchsorted(g_of, np.arange(NCORE * NWIN + 1))

    nlo = np.zeros(NCORE * NWIN, np.int64)
    nhi = np.zeros(NCORE * NWIN, np.int64)
    for g in range(NCORE * NWIN):
        s = ss[bounds[g]:bounds[g + 1]]
        nlo[g] = int((s < HALF).sum())
        nhi[g] = len(s) - nlo[g]
    # per-window tile counts (max over cores for SPMD uniformity)
    TLOs = [max(1, int(np.ceil(nlo[w::NWIN].max() / 128))) for w in range(NWIN)]
    THIs = [max(1, int(np.ceil(nhi[w::NWIN].max() / 128))) for w in range(NWIN)]
    TLOs = tuple(TLOs)
    THIs = tuple(THIs)
    Tmax = max(lo + hi for lo, hi in zip(TLOs, THIs))
    KWmax = max((lo + hi) * 2 * 8 for lo, hi in zip(TLOs, THIs))

    idxall = np.zeros((NCORE, NWIN, 128, KWmax), np.int16)
    rh = np.zeros((NCORE, NWIN, 128, Tmax, 4), np.float16)
    dl = np.zeros((NCORE, NWIN, 128, Tmax), np.float16)

    for g in range(NCORE * NWIN):
        c, w = divmod(g, NWIN)
        TLO, THI = TLOs[w], THIs[w]
        T = TLO + THI
        lo_e, hi_e = bounds[g], bounds[g + 1]
        s = ss[lo_e:hi_e]
        d_loc = (sd[lo_e:hi_e] - g * DSTW).astype(np.float32)
        r = sr[lo_e:hi_e]
        mlo = s < HALF

        # slot order: [lo edges, lo pads, hi edges, hi pads]
        n0, n1 = int(mlo.sum()), int((~mlo).sum())
        slots = T * 128
        sl_rel = np.zeros((slots, 2), np.float32)
        sl_dl = np.full(slots, 127.0, np.float32)
        sl_rel[:n0] = r[mlo]
        sl_dl[:n0] = d_loc[mlo]
        h0 = TLO * 128
        sl_rel[h0:h0 + n1] = r[~mlo]
        sl_dl[h0:h0 + n1] = d_loc[~mlo]

        idx_lo = s[mlo].astype(np.int16)
        idx_hi = (s[~mlo] - HALF).astype(np.int16)
        gd = np.zeros(slots, np.int16)
        gd[:n0] = (sd[lo_e:hi_e][mlo] - g * DSTW).astype(np.int16)
        gd[h0:h0 + n1] = (sd[lo_e:hi_e][~mlo] - g * DSTW).astype(np.int16)

        idxall[c, w, :, : TLO * 8] = _wrap_idx(idx_lo, TLO * 128)
        idxall[c, w, :, TLO * 8:(TLO + THI) * 8] = _wrap_idx(idx_hi, THI * 128)
        idxall[c, w, :, (TLO + THI) * 8:(TLO + THI + T) * 8] = _wrap_idx(gd, T * 128)

        # slot i -> (partition i%128, tile i//128)
        rh[c, w, :, :T, 0] = 1.0
        rh[c, w, :, :T, 1] = sl_rel[:, 0].reshape(T, 128).T.astype(np.float16)
        rh[c, w, :, :T, 2] = sl_rel[:, 1].reshape(T, 128).T.astype(np.float16)
        rh[c, w, :, :T, 3] = sl_dl.reshape(T, 128).T.astype(np.float16)
        dl[c, w, :, :T] = sl_dl.reshape(T, 128).T

    return TLOs, THIs, Tmax, KWmax, idxall, rh, dl


def _build_program(TLOs, THIs, Tmax, KWmax):
    key = (TLOs, THIs)
    if key in _PROGRAM_CACHE:
        return _PROGRAM_CACHE[key]

    import concourse.bass as bass
    import concourse.bacc as bacc
    import concourse.mybir as mybir
    import concourse.tile as tile

    f16 = mybir.dt.float16
    f32 = mybir.dt.float32
    i16 = mybir.dt.int16
    AF = mybir.ActivationFunctionType
    OP = mybir.AluOpType

    nc = bacc.Bacc("TRN2", target_bir_lowering=False, debug=False,
                   enable_asserts=True, num_devices=NCORE, num_swdge_queues=4)

    F16 = nc.dram_tensor("feat0", [NPAD, D], f16, kind="ExternalInput").ap()
    IDX = nc.dram_tensor("idxall", [NWIN, 128, KWmax], i16, kind="ExternalInput").ap()
    RH = nc.dram_tensor("rh", [NWIN, 128, Tmax, 4], f16, kind="ExternalInput").ap()
    DLOC = nc.dram_tensor("dloc", [NWIN, 128, Tmax], f16, kind="ExternalInput").ap()
    IDENT = nc.dram_tensor("ident", [128, 128], f16, kind="ExternalInput").ap()
    WCAT = nc.dram_tensor("wcat", [L, 128, 260], f16, kind="ExternalInput").ap()
    WSC = nc.dram_tensor("wsc", [L, 128, 2], f32, kind="ExternalInput").ap()
    WRB = nc.dram_tensor("wrb", [L, 2, 128, 128], f16, kind="ExternalInput").ap()
    IOTA = nc.dram_tensor("iota", [128, 128], f16, kind="ExternalInput").ap()
    OUT = nc.dram_tensor("out", [SHARD, D], f32, kind="ExternalOutput").ap()

    tabU = nc.dram_tensor("tabU", [NPAD, ROWE], f16, kind="Internal").ap()
    tabAD = nc.dram_tensor("tabAD", [SHARD_PAD, 128], f16, kind="Internal").ap()
    tabV = nc.dram_tensor("tabV", [SHARD_PAD, 128], f16, kind="Internal").ap()
    f1own = nc.dram_tensor("f1own", [SHARD_PAD, D], f16, kind="Internal").ap()
    f1full = nc.dram_tensor("f1full", [NPAD, D], f16, kind="Internal",
                            addr_space="Shared").ap()

    with tile.TileContext(nc) as tc:
        with tc.tile_pool(name="const", bufs=1) as cp, \
             tc.tile_pool(name="dense", bufs=3) as dp, \
             tc.tile_pool(name="psumd", bufs=2, space="PSUM") as pdp, \
             tc.tile_pool(name="psumw", bufs=2, space="PSUM") as pwp, \
             tc.tile_pool(name="psumdt", bufs=2, space="PSUM") as pdt, \
             tc.tile_pool(name="psuma", bufs=2, space="PSUM") as pap, \
             tc.tile_pool(name="gath", bufs=4) as gp, \
             tc.tile_pool(name="sel", bufs=8) as selp, \
             tc.tile_pool(name="dall", bufs=3) as dallp, \
             tc.tile_pool(name="small", bufs=8) as sp, \
             tc.tile_pool(name="fin", bufs=4) as fp:

            iota_t = cp.tile([128, 128], f16, tag="iota")
            nc.sync.dma_start(iota_t[:], IOTA[:])
            ident_t = cp.tile([128, 128], f16, tag="ident")
            nc.sync.dma_start(ident_t[:], IDENT[:])
            wcat_t = [cp.tile([128, 260], f16, tag=f"wcat{l}", name=f"wcat{l}")
                      for l in range(L)]
            wsc_t = [cp.tile([128, 2], f32, tag=f"wsc{l}", name=f"wsc{l}")
                     for l in range(L)]
            wr0_t = [cp.tile([128, 128], f16, tag=f"wr0{l}", name=f"wr0{l}")
                     for l in range(L)]
            wr1_t = [cp.tile([128, 128], f16, tag=f"wr1{l}", name=f"wr1{l}")
                     for l in range(L)]
            for l in range(L):
                nc.sync.dma_start(wcat_t[l][:], WCAT[l])
                nc.sync.dma_start(wsc_t[l][:], WSC[l])
                nc.sync.dma_start(wr0_t[l][:], WRB[l, 0])
                nc.sync.dma_start(wr1_t[l][:], WRB[l, 1])

            # zero f1own pad rows once
            zt = cp.tile([128, 128], f16, tag="zt")
            nc.vector.memset(zt[:], 0.0)
            for zk in range(SHARD, SHARD_PAD, 128):
                zn = min(128, SHARD_PAD - zk)
                nc.sync.dma_start(f1own[zk:zk + zn, :], zt[0:zn, :])

            pid = nc.scalar.partition_id()

            def dense_phase(l, featsrc):
                for ic in range(NDTILE // DCH):
                    ftT = dp.tile([128, DCH * 128], f16, tag="ftT",
                                  name=f"ftT{l}_{ic}")
                    nc.sync.dma_start(ftT[:], featsrc[ic * DCH * 128:(ic + 1) * DCH * 128, :],
                                      transpose=True)
                    stg = dp.tile([128, DCH * 130], f16, tag="stg", name=f"stg{l}_{ic}")
                    stgV = dp.tile([128, DCH * 128], f16, tag="stgV", name=f"stgV{l}_{ic}")
                    for j in range(DCH):
                        psd = pdp.tile([128, 260], f32, tag="psd", name=f"psd{l}_{ic}_{j}")
                        nc.tensor.matmul(psd[:], ftT[:, j * 128:(j + 1) * 128],
                                         wcat_t[l][:], start=True, stop=True)
                        nc.vector.tensor_copy(stg[:, j * 130:(j + 1) * 130], psd[:, 0:130])
                        nc.scalar.activation(stgV[:, j * 128:(j + 1) * 128],
                                             psd[:, 130:258], AF.Copy)
                    stg3 = stg[:].rearrange("p (j e) -> p j e", e=130)
                    stgV3 = stgV[:].rearrange("p (j e) -> p j e", e=128)
                    outU = tabU[ic * DCH * 128:(ic + 1) * DCH * 128, 0:130]
                    nc.scalar.dma_start(outU.rearrange("(j p) e -> p j e", p=128), stg3)
                    owner = ic // (SHARD_PAD // (DCH * 128))
                    local = (ic % (SHARD_PAD // (DCH * 128))) * DCH * 128
                    outAD = tabAD[local:local + DCH * 128, 0:2]
                    nc.scalar.dma_start(outAD.rearrange("(j p) e -> p j e", p=128),
                                        stg3[:, :, 128:130], cond=(pid == owner))
                    outV = tabV[local:local + DCH * 128, :]
                    nc.scalar.dma_start(outV.rearrange("(j p) e -> p j e", p=128),
                                        stgV3, cond=(pid == owner))

            def agg_phase(l):
                for w in range(NWIN):
                    TLO, THI = TLOs[w], THIs[w]
                    T = TLO + THI
                    KW = 2 * T * 8
                    It = sp.tile([128, KW], i16, tag="It", name=f"It{l}_{w}")
                    nc.sync.dma_start(It[:, 0:(TLO + THI) * 8],
                                      IDX[w, :, 0:(TLO + THI) * 8])
                    G = gp.tile([128, T * ROWE], f16, tag="G", name=f"G{l}_{w}")
                    G3 = G[:].rearrange("p (t e) -> p t e", e=ROWE)
                    nc.gpsimd.dma_gather(
                        out_ap=G3[:, 0:TLO, :], in_ap=tabU[0:HALF, :],
                        idxs_ap=It[:, 0:TLO * 8],
                        num_idxs=TLO * 128, num_idxs_reg=TLO * 128,
                        elem_size=ROWE, single_packet=False,
                        queue_num=(2 * w) % 4)
                    nc.gpsimd.dma_gather(
                        out_ap=G3[:, TLO:T, :], in_ap=tabU[HALF:NPAD, :],
                        idxs_ap=It[:, TLO * 8:(TLO + THI) * 8],
                        num_idxs=THI * 128, num_idxs_reg=THI * 128,
                        elem_size=ROWE, single_packet=False,
                        queue_num=(2 * w + 1) % 4)
                    # host streams (one, relx, rely, dloc) into pad cols of G
                    nc.scalar.dma_start(G3[:, :, 130:134], RH[w, :, 0:T, :])
                    DLt = sp.tile([128, T], f16, tag="DLt", name=f"DL{l}_{w}")
                    nc.scalar.dma_start(DLt[:], DLOC[w, :, 0:T])
                    adw = sp.tile([128, 1], f16, tag="adw", name=f"adw{l}_{w}")
                    nc.scalar.dma_start(adw[:], tabAD[w * DSTW:w * DSTW + 128, 1:2])
                    Vw = fp.tile([128, 128], f16, tag="Vw", name=f"Vw{l}_{w}")
                    nc.scalar.dma_start(Vw[:], tabV[w * DSTW:w * DSTW + 128, :])

                    # one-hot D_t + a_dst expansion via PE transpose
                    psA = pap.tile([128, T], f32, tag="psA", name=f"psA{l}_{w}")
                    Dall = dallp.tile([128, T * 128], f16, tag="Dall",
                                     name=f"Dall{l}_{w}")
                    Dall3 = Dall[:].rearrange("p (t e) -> p t e", e=128)
                    for t in range(T):
                        nc.vector.tensor_tensor(
                            Dall3[:, t, :], iota_t[:],
                            DLt[:, t:t + 1].to_broadcast([128, 128]),
                            OP.is_equal)
                        psDT = pdt.tile([128, 128], f16, tag="psDT",
                                        name=f"psDT{l}_{w}_{t}")
                        nc.tensor.transpose(psDT[:], Dall3[:, t, :], ident_t[:])
                        DTs = selp.tile([128, 128], f16, tag="DTs",
                                        name=f"DTs{l}_{w}_{t}")
                        if t % 2 == 0:
                            nc.scalar.activation(DTs[:], psDT[:], AF.Copy)
                        else:
                            nc.vector.tensor_copy(DTs[:], psDT[:])
                        nc.tensor.matmul(psA[:, t:t + 1], DTs[:], adw[:],
                                         start=True, stop=True)

                    # scores [128, T] fp32
                    sA = sp.tile([128, T], f32, tag="sA", name=f"sA{l}_{w}")
                    sB = sp.tile([128, T], f32, tag="sB", name=f"sB{l}_{w}")
                    EX = sp.tile([128, T], f32, tag="EX", name=f"EX{l}_{w}")
                    nc.vector.tensor_tensor(
                        sA[:], G3[:, 0:T, 131], wsc_t[l][:, 0:1].to_broadcast([128, T]),
                        OP.mult)
                    nc.gpsimd.tensor_tensor(
                        sB[:], G3[:, 0:T, 132], wsc_t[l][:, 1:2].to_broadcast([128, T]),
                        OP.mult)
                    nc.vector.tensor_tensor(sA[:], sA[:], sB[:], OP.add)
                    nc.vector.tensor_tensor(sA[:], sA[:], G3[:, 0:T, 128], OP.add)
                    nc.vector.tensor_tensor(sA[:], sA[:], psA[:, 0:T], OP.add)
                    nc.gpsimd.tensor_scalar_mul(sB[:], sA[:], 0.2)
                    nc.vector.tensor_tensor(sA[:], sA[:], sB[:], OP.max)
                    nc.scalar.activation(EX[:], sA[:], AF.Exp)

                    psW = pwp.tile([128, 134], f32, tag="psW", name=f"psW{l}_{w}")
                    for t in range(T):
                        RS = selp.tile([128, 134], f16, tag="RS", name=f"RS{l}_{w}_{t}")
                        if t % 2 == 0:
                            nc.scalar.activation(RS[:], G3[:, t, 0:134], AF.Copy,
                                                 scale=EX[:, t:t + 1])
                        else:
                            nc.vector.tensor_tensor(
                                RS[:], G3[:, t, 0:134],
                                EX[:, t:t + 1].to_broadcast([128, 134]), OP.mult)
                        nc.tensor.matmul(psW[:], Dall3[:, t, :], RS[:],
                                         start=(t == 0), stop=(t == T - 1))

                    # finalize (gpsimd must not touch PSUM)
                    sc3 = sp.tile([128, 3], f32, tag="sc3", name=f"sc3{l}_{w}")
                    nc.vector.tensor_copy(sc3[:], psW[:, 130:133])
                    den = sc3[:, 0:1]
                    qx = sc3[:, 1:2]
                    qy = sc3[:, 2:3]
                    c1 = sp.tile([128, 1], f32, tag="c1", name=f"c1{l}_{w}")
                    om = sp.tile([128, 1], f32, tag="om", name=f"om{l}_{w}")
                    rc = sp.tile([128, 1], f32, tag="rc", name=f"rc{l}_{w}")
                    nc.gpsimd.tensor_scalar(out=c1[:], in0=den, scalar1=0.0,
                                            scalar2=None, op0=OP.is_gt)
                    nc.gpsimd.tensor_scalar(out=om[:], in0=c1[:], scalar1=-1.0,
                                            scalar2=1.0, op0=OP.mult, op1=OP.add)
                    nc.gpsimd.tensor_tensor(om[:], om[:], den, OP.add)
                    nc.vector.reciprocal(rc[:], om[:])
                    nc.vector.tensor_tensor(rc[:], rc[:], c1[:], OP.mult)
                    t0 = fp.tile([128, 128], f32, tag="t0", name=f"t0{l}_{w}")
                    t1 = fp.tile([128, 128], f32, tag="t1", name=f"t1{l}_{w}")
                    nc.vector.tensor_tensor(
                        t0[:], wr0_t[l][:], qx.to_broadcast([128, 128]), OP.mult)
                    nc.gpsimd.tensor_tensor(
                        t1[:], wr1_t[l][:], qy.to_broadcast([128, 128]), OP.mult)
                    nc.vector.tensor_tensor(t0[:], t0[:], t1[:], OP.add)
                    nc.vector.tensor_tensor(t0[:], t0[:], psW[:, 0:128], OP.add)
                    nc.vector.tensor_tensor(
                        t0[:], t0[:], rc[:].to_broadcast([128, 128]), OP.mult)
                    nc.gpsimd.tensor_tensor(
                        t1[:], Vw[:], c1[:].to_broadcast([128, 128]), OP.mult)
                    nc.vector.tensor_tensor(t0[:], t0[:], t1[:], OP.add)
                    if l == 0:
                        ot = fp.tile([128, 128], f16, tag="ot0", name=f"ot{l}_{w}")
                        nc.scalar.activation(ot[:], t0[:], AF.Relu)
                        nc.sync.dma_start(f1own[w * DSTW:w * DSTW + DSTW, :],
                                          ot[0:DSTW, :])
                    else:
                        ot = fp.tile([128, 128], f32, tag="ot1", name=f"ot{l}_{w}")
                        nc.scalar.activation(ot[:], t0[:], AF.Relu)
                        nc.sync.dma_start(OUT[w * DSTW:w * DSTW + DSTW, :],
                                          ot[0:DSTW, :])

            dense_phase(0, F16)
            tc.strict_bb_all_engine_barrier()
            agg_phase(0)
            tc.strict_bb_all_engine_barrier()
            nc.gpsimd.collective_compute(
                "AllGather", mybir.AluOpType.bypass,
                replica_groups=[list(range(NCORE))],
                ins=[f1own[:]], outs=[f1full[:]])
            tc.strict_bb_all_engine_barrier()
            dense_phase(1, f1full)
            tc.strict_bb_all_engine_barrier()
            agg_phase(1)

    nc.compile()
    _PROGRAM_CACHE[key] = nc
    return nc


def _host_inputs(inputs, idxall, rh, dl):
    af = np.asarray(inputs["actor_features"], np.float32)
    W_att = np.asarray(inputs["W_att"], np.float32)
    W_emb = np.asarray(inputs["W_emb"], np.float32)

    F16 = np.zeros((NPAD, D), np.float16)
    a = np.arange(N_ACTORS)
    F16[_rho(a)] = af.astype(np.float16)

    WCAT = np.zeros((L, 128, 260), np.float16)
    WSC = np.zeros((L, 128, 2), np.float32)
    WRB = np.zeros((L, 2, 128, 128), np.float16)
    for l in range(L):
        WCAT[l, :, 0:128] = W_emb[l][:, 0:128].T.astype(np.float16)
        WCAT[l, :, 128] = W_att[l][0:128].astype(np.float16)
        WCAT[l, :, 129] = W_att[l][130:258].astype(np.float16)
        WCAT[l, :, 130:258] = W_emb[l][:, 130:258].T.astype(np.float16)
        WSC[l, :, 0] = W_att[l][128]
        WSC[l, :, 1] = W_att[l][129]
        WRB[l, 0] = np.tile(W_emb[l][:, 128].astype(np.float16), (128, 1))
        WRB[l, 1] = np.tile(W_emb[l][:, 129].astype(np.float16), (128, 1))
    IOTA = np.tile(np.arange(128, dtype=np.float16), (128, 1))
    IDENT = np.eye(128, dtype=np.float16)

    in_maps = []
    for c in range(NCORE):
        in_maps.append({
            "feat0": F16,
            "idxall": idxall[c],
            "rh": rh[c],
            "dloc": dl[c],
            "wcat": WCAT,
            "wsc": WSC,
            "wrb": WRB,
            "iota": IOTA,
            "ident": IDENT,
        })
    return in_maps


def kernel(**inputs):
    from concourse import bass_utils

    TLOs, THIs, Tmax, KWmax, idxall, rh, dl = _build_plan(
        inputs["edge_src_idx"], inputs["edge_dst_idx"], inputs["edge_dist_rel"])
    nc = _build_program(TLOs, THIs, Tmax, KWmax)
    in_maps = _host_inputs(inputs, idxall, rh, dl)

    trace = os.environ.get("KERNEL_TRACE", "0") == "1"
    res = bass_utils.run_bass_kernel_spmd(
        nc, in_maps, core_ids=list(range(NCORE)), trace=trace)
    if trace and res.exec_time_ns is not None:
        print(f"HW exec time: {res.exec_time_ns} ns")

    out = np.concatenate([res.results[c]["out"] for c in range(NCORE)], axis=0)
    return out.astype(np.float32)



# revision 3
# speedup vs baseline: 1.5800x; 1.5800x over previous
"""Trainium2 Bass kernel for nn_Actor2ActorAttention (2-layer edge-attention GNN).

v2 strategy (single SPMD launch on 8 NeuronCores):
  - Host: assign every actor a "slot" = (core, window, partition) so that each
    of the 49 windows per core owns 128 dst actors (one per partition).
    Actors are classified lo/hi by core (cores 0-4 -> table rows < 31360,
    int16-gatherable; cores 5-7 via a second gather with base offset), and
    dsts are grouped into windows by similar (lo-edge-count, hi-edge-count)
    so per-window tile counts stay near the mean degree.
  - Device, per layer:
      dense phase: feat @ WCAT -> per-actor 256B table rows
        [U[0:127] | a_src] (tabU, gathered by edge src), plus own-shard
        tables a_dst (tabAD) and V (tabV).
      agg phase, per window: two dma_gathers fetch each edge's src row into
        G[p=dst partition, t, 128]; scores = a_src + rel-term + a_dst[p]
        (all per-partition broadcasts -- no transposes, no PSUM);
        EX = exp(leaky_relu); RS[:, t, :] = [G*EX | relx*EX | rely*EX | EX];
        pairwise tree-add over t gives P | qx | qy | den per dst;
        U's 128th component is recovered post-sum via v' = solve(M^T, Wsrc[127])
        (row127 = v' . row holds by linearity through the weighted sum).
        Finalize: agg = (P + qx*Wr0 + qy*Wr1)/den + [den>0]*V, relu.
  - AllGather exchanges layer-1 features between layers.
"""

import os
import sys

sys.path.insert(0, "/opt/trn_rl_repo")

import numpy as np

# ---------------- problem constants (hardcoded per spec) ----------------
N_ACTORS = 50000
D = 128
L = 2
E = 800000
NCORE = 8
WPC = 49                        # windows per core
SHARD = WPC * 128               # 6272 slots per core
NPAD = NCORE * SHARD            # 50176 table rows
NLO_CORES = 5
LO_ROWS = NLO_CORES * SHARD     # 31360; rows < LO_ROWS use gather 1
HI_ROWS = NPAD - LO_ROWS        # 18816; gather 2 base offset LO_ROWS
DCH = 7                         # dense tiles per transpose-load chunk
NDTILE = NPAD // 128            # 392 dense tiles

_PROGRAM_CACHE = {}


def _wrap_idx(vals, n):
    """Gather index layout: element i at [i%16, i//16], replicated to all
    8 groups of 16 partitions -> [128, n//16] int16."""
    a = np.zeros(n, np.int16)
    a[: len(vals)] = vals
    return np.tile(a.reshape(-1, 16).T, (8, 1))


def _build_plan(src, dst, rel, W_att):
    src = np.asarray(src).astype(np.int64)
    dst = np.asarray(dst).astype(np.int64)
    rel = np.asarray(rel).astype(np.float64)
    W_att = np.asarray(W_att).astype(np.float64)

    deg = np.bincount(dst, minlength=N_ACTORS)

    # phase A: fix the lo/hi actor split (core 0-4 vs 5-7) by dealing
    # in-degree-sorted actors round-robin over the 8 cores.
    order0 = np.argsort(-deg, kind="stable")
    core0 = np.empty(N_ACTORS, np.int64)
    core0[order0] = np.arange(N_ACTORS) % NCORE
    is_lo_actor = core0 < NLO_CORES

    # per-dst lo/hi in-edge counts (now fixed regardless of re-dealing
    # within the lo / hi core groups)
    el = is_lo_actor[src]
    lcnt = np.bincount(dst[el], minlength=N_ACTORS)
    hcnt = np.bincount(dst[~el], minlength=N_ACTORS)

    # phase B: within each group, sort dsts by (l, h) desc and deal into
    # (window, core, partition); windows collect dsts of similar (l, h).
    slot_of = np.full(N_ACTORS, -1, np.int64)
    for grp_mask, cbase, ncg in ((is_lo_actor, 0, NLO_CORES),
                                 (~is_lo_actor, NLO_CORES, NCORE - NLO_CORES)):
        grp = np.where(grp_mask)[0]
        o = np.lexsort((-hcnt[grp], -lcnt[grp]))
        g = grp[o]
        i = np.arange(len(g))
        w = i // (ncg * 128)
        j = i % (ncg * 128)
        c = cbase + j % ncg
        p = j // ncg
        slot_of[g] = c * SHARD + w * 128 + p

    # per-edge placement
    es = slot_of[dst]
    ec = es // SHARD
    ew = (es % SHARD) // 128
    ep = es % 128
    ehi = (~is_lo_actor[src]).astype(np.int64)
    erow = slot_of[src]                   # table row of src

    # per (core, window, partition) lo/hi counts -> TLO/THI per window
    cnt = np.zeros((NCORE, WPC, 128, 2), np.int64)
    np.add.at(cnt, (ec, ew, ep, ehi), 1)
    TLOs = tuple(int(max(1, cnt[:, w, :, 0].max())) for w in range(WPC))
    THIs = tuple(int(max(1, cnt[:, w, :, 1].max())) for w in range(WPC))
    Tmax = max(lo + hi for lo, hi in zip(TLOs, THIs))
    KWmax = max((lo + hi) * 8 for lo, hi in zip(TLOs, THIs))

    # rank of each edge within its (core, window, partition, side) bucket
    okey = np.lexsort((np.arange(E), ehi, ep, ew, ec))
    sc, sw, sp, sh = ec[okey], ew[okey], ep[okey], ehi[okey]
    bucket = ((sc * WPC + sw) * 128 + sp) * 2 + sh
    starts = np.searchsorted(bucket, np.arange(NCORE * WPC * 128 * 2 + 1))
    rank_sorted = np.arange(E) - starts[bucket]
    rank = np.empty(E, np.int64)
    rank[okey] = rank_sorted

    # pad rows: any dummy slot in each region (slot with no actor)
    used = np.zeros(NPAD, bool)
    used[slot_of] = True
    lo_pad = int(np.where(~used[:LO_ROWS])[0][-1])
    hi_pad = int(np.where(~used[LO_ROWS:])[0][-1])  # relative to LO_ROWS

    # fill IDX / AUX host streams
    idxall = np.zeros((NCORE, WPC, 128, KWmax), np.int16)
    aux = np.zeros((L, NCORE, WPC, 128, Tmax, 4), np.float16)
    aux[:, :, :, :, :, 3] = -20.0         # rt pad => exp flushes to 0
    aux[:, :, :, :, :, 2] = 1.0

    # slot position of each edge inside its window gather streams
    et = np.where(ehi == 0, rank, rank)   # rank within side
    rts = [W_att[l][128] * rel[:, 0] + W_att[l][129] * rel[:, 1]
           for l in range(L)]

    TLO_arr = np.array(TLOs)[ew]
    # scatter per-edge values into aux (vectorized)
    t_slot = np.where(ehi == 0, et, TLO_arr + et)
    aux[0, ec, ew, ep, t_slot, 0] = rel[:, 0].astype(np.float16)
    aux[0, ec, ew, ep, t_slot, 1] = rel[:, 1].astype(np.float16)
    aux[0, ec, ew, ep, t_slot, 3] = rts[0].astype(np.float16)
    aux[1, ec, ew, ep, t_slot, 0] = rel[:, 0].astype(np.float16)
    aux[1, ec, ew, ep, t_slot, 1] = rel[:, 1].astype(np.float16)
    aux[1, ec, ew, ep, t_slot, 3] = rts[1].astype(np.float16)

    # gather index streams
    for c in range(NCORE):
        for w in range(WPC):
            TLO, THI = TLOs[w], THIs[w]
            m = (ec == c) & (ew == w)
            lo_idx = np.full(TLO * 128, lo_pad, np.int64)
            hi_idx = np.full(THI * 128, hi_pad, np.int64)
            ml = m & (ehi == 0)
            mh = m & (ehi == 1)
            lo_idx[rank[ml] * 128 + ep[ml]] = erow[ml]
            hi_idx[rank[mh] * 128 + ep[mh]] = erow[mh] - LO_ROWS
            idxall[c, w, :, 0:TLO * 8] = _wrap_idx(lo_idx.astype(np.int16),
                                                   TLO * 128)
            idxall[c, w, :, TLO * 8:(TLO + THI) * 8] = _wrap_idx(
                hi_idx.astype(np.int16), THI * 128)

    return TLOs, THIs, Tmax, KWmax, idxall, aux, slot_of


def _build_program(TLOs, THIs, Tmax, KWmax):
    key = (TLOs, THIs)
    if key in _PROGRAM_CACHE:
        return _PROGRAM_CACHE[key]

    import concourse.bass as bass
    import concourse.bacc as bacc
    import concourse.mybir as mybir
    import concourse.tile as tile

    f16 = mybir.dt.float16
    f32 = mybir.dt.float32
    i16 = mybir.dt.int16
    AF = mybir.ActivationFunctionType
    OP = mybir.AluOpType

    nc = bacc.Bacc("TRN2", target_bir_lowering=False, debug=False,
                   enable_asserts=True, num_devices=NCORE, num_swdge_queues=4)

    F16 = nc.dram_tensor("feat0", [NPAD, D], f16, kind="ExternalInput").ap()
    IDX = nc.dram_tensor("idxall", [WPC, 128, KWmax], i16,
                         kind="ExternalInput").ap()
    AUX = nc.dram_tensor("aux", [L, WPC, 128, Tmax, 4], f16,
                         kind="ExternalInput").ap()
    WCAT = nc.dram_tensor("wcat", [L, 128, 260], f16, kind="ExternalInput").ap()
    WRB = nc.dram_tensor("wrb", [L, 2, 128, 128], f16, kind="ExternalInput").ap()
    VP = nc.dram_tensor("vp", [L, 128, 128], f32, kind="ExternalInput").ap()
    OUT = nc.dram_tensor("out", [SHARD, D], f32, kind="ExternalOutput").ap()

    tabU = nc.dram_tensor("tabU", [NPAD, 128], f16, kind="Internal").ap()
    tabAD = nc.dram_tensor("tabAD", [SHARD, 1], f32, kind="Internal").ap()
    tabV = nc.dram_tensor("tabV", [SHARD, 128], f16, kind="Internal").ap()
    f1own = nc.dram_tensor("f1own", [SHARD, D], f16, kind="Internal").ap()
    f1full = nc.dram_tensor("f1full", [NPAD, D], f16, kind="Internal",
                            addr_space="Shared").ap()

    with tile.TileContext(nc) as tc:
        with tc.tile_pool(name="const", bufs=1) as cp, \
             tc.tile_pool(name="dense", bufs=3) as dp, \
             tc.tile_pool(name="psumd", bufs=4, space="PSUM") as pdp, \
             tc.tile_pool(name="gath", bufs=3) as gp, \
             tc.tile_pool(name="rs", bufs=2) as rp, \
             tc.tile_pool(name="small", bufs=8) as sp, \
             tc.tile_pool(name="fin", bufs=4) as fp:

            wcat_t = [cp.tile([128, 260], f16, tag=f"wcat{l}", name=f"wcat{l}")
                      for l in range(L)]
            wr0_t = [cp.tile([128, 128], f16, tag=f"wr0{l}", name=f"wr0{l}")
                     for l in range(L)]
            wr1_t = [cp.tile([128, 128], f16, tag=f"wr1{l}", name=f"wr1{l}")
                     for l in range(L)]
            vp_t = [cp.tile([128, 128], f32, tag=f"vp{l}", name=f"vp{l}")
                    for l in range(L)]
            for l in range(L):
                nc.sync.dma_start(wcat_t[l][:], WCAT[l])
                nc.sync.dma_start(wr0_t[l][:], WRB[l, 0])
                nc.sync.dma_start(wr1_t[l][:], WRB[l, 1])
                nc.sync.dma_start(vp_t[l][:], VP[l])

            pid = nc.scalar.partition_id()

            def dense_phase(l, featsrc):
                for ic in range(NDTILE // DCH):
                    ftT = dp.tile([128, DCH * 128], f16, tag="ftT",
                                  name=f"ftT{l}_{ic}")
                    nc.sync.dma_start(
                        ftT[:],
                        featsrc[ic * DCH * 128:(ic + 1) * DCH * 128, :],
                        transpose=True)
                    stgU = dp.tile([128, DCH * 128], f16, tag="stgU",
                                   name=f"stgU{l}_{ic}")
                    stgV = dp.tile([128, DCH * 128], f16, tag="stgV",
                                   name=f"stgV{l}_{ic}")
                    stgA = dp.tile([128, DCH], f32, tag="stgA",
                                   name=f"stgA{l}_{ic}")
                    for j in range(DCH):
                        psd = pdp.tile([128, 260], f32, tag="psd",
                                       name=f"psd{l}_{ic}_{j}")
                        nc.tensor.matmul(psd[:], ftT[:, j * 128:(j + 1) * 128],
                                         wcat_t[l][:], start=True, stop=True)
                        if j % 2 == 0:
                            nc.vector.tensor_copy(
                                stgU[:, j * 128:(j + 1) * 128], psd[:, 0:128])
                            nc.scalar.activation(
                                stgV[:, j * 128:(j + 1) * 128],
                                psd[:, 129:257], AF.Copy)
                        else:
                            nc.scalar.activation(
                                stgU[:, j * 128:(j + 1) * 128],
                                psd[:, 0:128], AF.Copy)
                            nc.vector.tensor_copy(
                                stgV[:, j * 128:(j + 1) * 128],
                                psd[:, 129:257])
                        nc.vector.tensor_copy(stgA[:, j:j + 1],
                                              psd[:, 128:129])
                    stgU3 = stgU[:].rearrange("p (j e) -> p j e", e=128)
                    stgV3 = stgV[:].rearrange("p (j e) -> p j e", e=128)
                    rows = slice(ic * DCH * 128, (ic + 1) * DCH * 128)
                    nc.scalar.dma_start(
                        tabU[rows, :].rearrange("(j p) e -> p j e", p=128),
                        stgU3)
                    owner = ic // (WPC // DCH)
                    local = (ic % (WPC // DCH)) * DCH * 128
                    nc.scalar.dma_start(
                        tabV[local:local + DCH * 128, :].rearrange(
                            "(j p) e -> p j e", p=128),
                        stgV3, cond=(pid == owner))
                    nc.scalar.dma_start(
                        tabAD[local:local + DCH * 128, 0:1].rearrange(
                            "(j p) e -> p j e", p=128),
                        stgA[:].rearrange("p (j e) -> p j e", e=1),
                        cond=(pid == owner))

            def agg_phase(l):
                for w in range(WPC):
                    TLO, THI = TLOs[w], THIs[w]
                    T = TLO + THI
                    It = sp.tile([128, KWmax], i16, tag="It", name=f"It{l}_{w}")
                    nc.sync.dma_start(It[:, 0:T * 8], IDX[w, :, 0:T * 8])
                    Ax = sp.tile([128, Tmax * 4], f16, tag="Ax",
                                 name=f"Ax{l}_{w}")
                    Ax3 = Ax[:].rearrange("p (t e) -> p t e", e=4)
                    nc.sync.dma_start(Ax3[:, 0:T, :], AUX[l, w, :, 0:T, :])
                    adw = sp.tile([128, 1], f32, tag="adw", name=f"adw{l}_{w}")
                    nc.scalar.dma_start(adw[:], tabAD[w * 128:(w + 1) * 128,
                                                      0:1])
                    Vw = fp.tile([128, 128], f16, tag="Vw", name=f"Vw{l}_{w}")
                    nc.scalar.dma_start(Vw[:], tabV[w * 128:(w + 1) * 128, :])

                    G = gp.tile([128, Tmax * 128], f16, tag="G",
                                name=f"G{l}_{w}")
                    G3 = G[:].rearrange("p (t e) -> p t e", e=128)
                    nc.gpsimd.dma_gather(
                        out_ap=G3[:, 0:TLO, :], in_ap=tabU[0:32768, :],
                        idxs_ap=It[:, 0:TLO * 8],
                        num_idxs=TLO * 128, num_idxs_reg=TLO * 128,
                        elem_size=128, single_packet=False,
                        queue_num=(2 * w) % 4)
                    nc.gpsimd.dma_gather(
                        out_ap=G3[:, TLO:T, :], in_ap=tabU[LO_ROWS:NPAD, :],
                        idxs_ap=It[:, TLO * 8:T * 8],
                        num_idxs=THI * 128, num_idxs_reg=THI * 128,
                        elem_size=128, single_packet=False,
                        queue_num=(2 * w + 1) % 4)

                    # scores: s = a_src + rel_term + a_dst[p]; lrelu; exp
                    sA = sp.tile([128, Tmax], f32, tag="sA", name=f"sA{l}_{w}")
                    sB = sp.tile([128, Tmax], f32, tag="sB", name=f"sB{l}_{w}")
                    EX = sp.tile([128, Tmax], f16, tag="EX", name=f"EX{l}_{w}")
                    nc.vector.tensor_tensor(sA[:, 0:T], G3[:, 0:T, 127],
                                            Ax3[:, 0:T, 3], OP.add)
                    nc.gpsimd.tensor_tensor(
                        sA[:, 0:T], sA[:, 0:T],
                        adw[:, 0:1].to_broadcast([128, T]), OP.add)
                    nc.scalar.activation(sB[:, 0:T], sA[:, 0:T], AF.Copy,
                                         scale=0.2)
                    nc.vector.tensor_tensor(sA[:, 0:T], sA[:, 0:T], sB[:, 0:T],
                                            OP.max)
                    nc.scalar.activation(EX[:, 0:T], sA[:, 0:T], AF.Exp)

                    # RS = [G*EX | relx*EX | rely*EX | EX]; tree-add over t
                    RS = rp.tile([128, Tmax * 132], f16, tag="RS",
                                 name=f"RS{l}_{w}")
                    RS3 = RS[:].rearrange("p (t e) -> p t e", e=132)
                    nc.vector.tensor_tensor(
                        RS3[:, 0:T, 0:128], G3[:, 0:T, 0:128],
                        EX[:, 0:T].to_broadcast([128, T, 128]), OP.mult)
                    nc.vector.tensor_tensor(
                        RS3[:, 0:T, 128:131], Ax3[:, 0:T, 0:3],
                        EX[:, 0:T].to_broadcast([128, T, 3]), OP.mult)
                    cur = T
                    while cur > 1:
                        nxt = (cur + 1) // 2
                        k = cur - nxt
                        nc.vector.tensor_tensor(
                            RS3[:, 0:k, 0:131], RS3[:, 0:k, 0:131],
                            RS3[:, nxt:cur, 0:131], OP.add)
                        cur = nxt

                    # recover payload col 127: u127 = v' . P_row
                    tmp = fp.tile([128, 128], f32, tag="tmp",
                                  name=f"tmp{l}_{w}")
                    u127 = sp.tile([128, 1], f32, tag="u127",
                                   name=f"u127{l}_{w}")
                    nc.vector.tensor_tensor(tmp[:], RS3[:, 0, 0:128],
                                            vp_t[l][:], OP.mult)
                    nc.vector.tensor_reduce(u127[:], tmp[:],
                                            mybir.AxisListType.X, OP.add)
                    nc.vector.tensor_copy(RS3[:, 0, 127:128], u127[:])

                    # finalize
                    den = RS3[:, 0, 130:131]
                    qx = RS3[:, 0, 128:129]
                    qy = RS3[:, 0, 129:130]
                    c1 = sp.tile([128, 1], f32, tag="c1", name=f"c1{l}_{w}")
                    om = sp.tile([128, 1], f32, tag="om", name=f"om{l}_{w}")
                    rc = sp.tile([128, 1], f32, tag="rc", name=f"rc{l}_{w}")
                    nc.gpsimd.tensor_scalar(out=c1[:], in0=den, scalar1=0.0,
                                            scalar2=None, op0=OP.is_gt)
                    nc.gpsimd.tensor_scalar(out=om[:], in0=c1[:], scalar1=-1.0,
                                            scalar2=1.0, op0=OP.mult,
                                            op1=OP.add)
                    nc.gpsimd.tensor_tensor(om[:], om[:], den, OP.add)
                    nc.vector.reciprocal(rc[:], om[:])
                    nc.vector.tensor_tensor(rc[:], rc[:], c1[:], OP.mult)
                    t0 = fp.tile([128, 128], f32, tag="t0", name=f"t0{l}_{w}")
                    t1 = fp.tile([128, 128], f32, tag="t1", name=f"t1{l}_{w}")
                    nc.vector.tensor_tensor(
                        t0[:], wr0_t[l][:], qx.to_broadcast([128, 128]),
                        OP.mult)
                    nc.gpsimd.tensor_tensor(
                        t1[:], wr1_t[l][:], qy.to_broadcast([128, 128]),
                        OP.mult)
                    nc.vector.tensor_tensor(t0[:], t0[:], t1[:], OP.add)
                    nc.vector.tensor_tensor(t0[:], t0[:], RS3[:, 0, 0:128],
                                            OP.add)
                    nc.vector.tensor_tensor(
                        t0[:], t0[:], rc[:].to_broadcast([128, 128]), OP.mult)
                    nc.gpsimd.tensor_tensor(
                        t1[:], Vw[:], c1[:].to_broadcast([128, 128]), OP.mult)
                    nc.vector.tensor_tensor(t0[:], t0[:], t1[:], OP.add)
                    if l == 0:
                        ot = fp.tile([128, 128], f16, tag="ot0",
                                     name=f"ot{l}_{w}")
                        nc.scalar.activation(ot[:], t0[:], AF.Relu)
                        nc.sync.dma_start(f1own[w * 128:(w + 1) * 128, :],
                                          ot[:])
                    else:
                        ot = fp.tile([128, 128], f32, tag="ot1",
                                     name=f"ot{l}_{w}")
                        nc.scalar.activation(ot[:], t0[:], AF.Relu)
                        nc.sync.dma_start(OUT[w * 128:(w + 1) * 128, :], ot[:])

            dense_phase(0, F16)
            tc.strict_bb_all_engine_barrier()
            agg_phase(0)
            tc.strict_bb_all_engine_barrier()
            nc.gpsimd.collective_compute(
                "AllGather", mybir.AluOpType.bypass,
                replica_groups=[list(range(NCORE))],
                ins=[f1own[:]], outs=[f1full[:]])
            tc.strict_bb_all_engine_barrier()
            dense_phase(1, f1full)
            tc.strict_bb_all_engine_barrier()
            agg_phase(1)

    nc.compile()
    _PROGRAM_CACHE[key] = nc
    return nc


def _host_inputs(inputs, idxall, aux, slot_of):
    af = np.asarray(inputs["actor_features"], np.float32)
    W_att = np.asarray(inputs["W_att"], np.float64)
    W_emb = np.asarray(inputs["W_emb"], np.float64)

    F16 = np.zeros((NPAD, D), np.float16)
    F16[slot_of] = af.astype(np.float16)

    WCAT = np.zeros((L, 128, 260), np.float16)
    WRB = np.zeros((L, 2, 128, 128), np.float16)
    VP = np.zeros((L, 128, 128), np.float32)
    for l in range(L):
        Wsrc = W_emb[l][:, 0:128]          # [out, in]
        wa_s = W_att[l][0:128]
        wa_d = W_att[l][130:258]
        WCAT[l, :, 0:127] = Wsrc[0:127].T.astype(np.float16)
        WCAT[l, :, 127] = wa_s.astype(np.float16)
        WCAT[l, :, 128] = wa_d.astype(np.float16)
        WCAT[l, :, 129:257] = W_emb[l][:, 130:258].T.astype(np.float16)
        WRB[l, 0] = np.tile(W_emb[l][:, 128].astype(np.float16), (128, 1))
        WRB[l, 1] = np.tile(W_emb[l][:, 129].astype(np.float16), (128, 1))
        M = np.concatenate([Wsrc[0:127], wa_s[None]], 0)
        vprime = np.linalg.solve(M.T, Wsrc[127])
        VP[l] = np.tile(vprime.astype(np.float32), (128, 1))

    in_maps = []
    for c in range(NCORE):
        in_maps.append({
            "feat0": F16,
            "idxall": idxall[c],
            "aux": aux[:, c],
            "wcat": WCAT,
            "wrb": WRB,
            "vp": VP,
        })
    return in_maps


def kernel(**inputs):
    from concourse import bass_utils

    TLOs, THIs, Tmax, KWmax, idxall, aux, slot_of = _build_plan(
        inputs["edge_src_idx"], inputs["edge_dst_idx"],
        inputs["edge_dist_rel"], inputs["W_att"])
    nc = _build_program(TLOs, THIs, Tmax, KWmax)
    in_maps = _host_inputs(inputs, idxall, aux, slot_of)

    trace = os.environ.get("KERNEL_TRACE", "0") == "1"
    res = bass_utils.run_bass_kernel_spmd(
        nc, in_maps, core_ids=list(range(NCORE)), trace=trace)
    if trace and res.exec_time_ns is not None:
        print(f"HW exec time: {res.exec_time_ns} ns")

    allout = np.concatenate([res.results[c]["out"] for c in range(NCORE)],
                            axis=0)
    return allout[slot_of].astype(np.float32)
